# revision 6
# baseline (speedup 1.0000x reference)
"""BatchTreeEncoder Trainium2 kernel (8 NeuronCores, data-parallel over trees).

Math (verified equal to the reference up to fp32 rounding):
  - The reference's child_attention(h_last)[g,i] reduces exactly to
        h_last[g,i] * sigmoid(v[g,i] - u0 - ln(S-1))
    where v = tanh(tanh(h_last @ sent_weight + sent_bias) @ context_weight)
    and u0 = tanh(tanh(sent_bias) @ context_weight), because ch[g,c,i,:] is
    diagonal in (c,i): the softmax row has one data term and S-1 constants.
  - Only the last sibling of each 4-child group feeds the parent GRU, and the
    final max only reads those same rows -> only nodes with index ==3 (mod 4)
    at levels 1..6 (plus all 32 roots) affect the output. That is 1/4 of the
    tree. Within the kept arrays the child of kept-node i is kept-node 4i+3.

Layout: everything lives as [128 H-rows, chunk, nodes] (two 128-row chunks of
H=256). Embedding rows are gathered on-device with indirect DMA (nodes on
partitions), PE-transposed to [E, nodes], and fed as matmul rhs.
"""

import math

import numpy as np

BS, A, D = 32, 4, 6
V, E, H = 50000, 256, 256
LEVEL_SIZES = [BS * A**d for d in range(D + 1)]
OFFSETS = np.concatenate([[0], np.cumsum(LEVEL_SIZES)]).astype(np.int64)

NCORES = 8
TPC = BS // NCORES  # trees per core = 4

# per-core needed-node counts per level (level 0 counts all roots)
CNT = {d: (TPC if d == 0 else TPC * 4 ** (d - 1)) for d in range(D + 1)}
# token gather columns per level (padded to multiples of 128 rows)
KCOLS = {d: max(1, CNT[d] // 128) for d in range(D + 1)}
COL_ORDER = [6, 5, 4, 3, 2, 1, 0]
COL_OFF = {}
_off = 0
for _d in COL_ORDER:
    COL_OFF[_d] = _off
    _off += KCOLS[_d]
TOK_COLS = _off  # 46

TILE_N = 256  # node-tile width
HC = 2        # H chunks of 128
GC = 6        # gate chunks of 128 (3H = 768)

_PROG = None


def _needed_global_indices(core):
    """Global positions in `tokens` of this core's needed nodes, per level."""
    trees = range(TPC * core, TPC * core + TPC)
    out = {}
    out[0] = np.array([int(OFFSETS[0]) + t for t in trees], dtype=np.int64)
    for d in range(1, D + 1):
        blk = 4**d
        idx = []
        for t in trees:
            base = int(OFFSETS[d]) + t * blk
            idx.append(base + 4 * np.arange(4 ** (d - 1), dtype=np.int64) + 3)
        out[d] = np.concatenate(idx)
    return out


def _token_plane(tokens, core):
    """[128, TOK_COLS] int32: idx[p, j] = token of node n = j*128 + p."""
    need = _needed_global_indices(core)
    plane = np.zeros((128, TOK_COLS), dtype=np.int32)
    for d in COL_ORDER:
        toks = tokens[need[d]].astype(np.int32)
        k = KCOLS[d]
        pad = np.zeros(k * 128, dtype=np.int32)
        pad[: len(toks)] = toks
        plane[:, COL_OFF[d]:COL_OFF[d] + k] = pad.reshape(k, 128).T
    return plane


def _build_program():
    import concourse.bacc as bacc
    import concourse.bass as bass
    import concourse.mybir as mybir
    import concourse.tile as tile
    from concourse.masks import make_identity

    f32 = mybir.dt.float32
    i32 = mybir.dt.int32
    Alu = mybir.AluOpType
    Act = mybir.ActivationFunctionType

    nc = bacc.Bacc("TRN2", target_bir_lowering=False, debug=False)

    toks_d = nc.dram_tensor("toks", [128, TOK_COLS], i32, kind="ExternalInput")
    emb_d = nc.dram_tensor("emb", [V, E], f32, kind="ExternalInput")
    wih_d = nc.dram_tensor("wihT", [E, 3 * H], f32, kind="ExternalInput")
    whh_d = nc.dram_tensor("whhT", [H, 3 * H], f32, kind="ExternalInput")
    bih_d = nc.dram_tensor("bih", [3 * H], f32, kind="ExternalInput")
    bhh_d = nc.dram_tensor("bhh", [3 * H], f32, kind="ExternalInput")
    sw_d = nc.dram_tensor("sw", [H, H], f32, kind="ExternalInput")
    sb_d = nc.dram_tensor("sb", [H], f32, kind="ExternalInput")
    cw_d = nc.dram_tensor("cw", [H, 1], f32, kind="ExternalInput")
    out_d = nc.dram_tensor("out", [HC, 128, TPC], f32, kind="ExternalOutput")

    with tile.TileContext(nc) as tc:
        with (
            tc.tile_pool(name="const", bufs=1) as cpool,
            tc.tile_pool(name="hbuf", bufs=1) as hpool,
            tc.tile_pool(name="xbuf", bufs=1) as xlpool,
            tc.tile_pool(name="gst", bufs=3) as gpool,
            tc.tile_pool(name="xst", bufs=3) as xpool,
            tc.tile_pool(name="work", bufs=3) as wpool,
            tc.tile_pool(name="psA", bufs=2, space="PSUM") as psA,
            tc.tile_pool(name="psG", bufs=2, space="PSUM") as psG,
        ):
            # ---- constants / weights ----
            toks = cpool.tile([128, TOK_COLS], i32, tag="toks")
            nc.sync.dma_start(toks[:], toks_d[:])
            wih = cpool.tile([128, HC, 3 * H], f32, tag="wih")
            nc.sync.dma_start(wih[:], wih_d[:].rearrange("(kc p) g -> p kc g", p=128))
            whh = cpool.tile([128, HC, 3 * H], f32, tag="whh")
            nc.sync.dma_start(whh[:], whh_d[:].rearrange("(kc p) g -> p kc g", p=128))
            sw = cpool.tile([128, HC, H], f32, tag="sw")
            nc.sync.dma_start(sw[:], sw_d[:].rearrange("(kc p) m -> p kc m", p=128))
            cw = cpool.tile([128, HC], f32, tag="cw")
            nc.sync.dma_start(cw[:], cw_d[:].rearrange("(kc p) o -> p (kc o)", p=128))
            bih = cpool.tile([128, GC], f32, tag="bih")
            nc.sync.dma_start(bih[:], bih_d[:].rearrange("(c p) -> p c", p=128))
            bhh = cpool.tile([128, GC], f32, tag="bhh")
            nc.sync.dma_start(bhh[:], bhh_d[:].rearrange("(c p) -> p c", p=128))
            bsum = cpool.tile([128, GC], f32, tag="bsum")
            nc.vector.tensor_add(bsum[:], bih[:], bhh[:])
            sbc = cpool.tile([128, HC], f32, tag="sbc")
            nc.sync.dma_start(sbc[:], sb_d[:].rearrange("(c p) -> p c", p=128))
            ident = cpool.tile([128, 128], f32, tag="ident")
            make_identity(nc, ident[:])
            ones1 = cpool.tile([1, 128], f32, tag="ones1")
            nc.vector.memset(ones1[:], 1.0)

            # u0 = tanh(tanh(sent_bias) @ cw); gate biases -(u0 + ln(S-1))
            sbrow = cpool.tile([1, H], f32, tag="sbrow")
            nc.sync.dma_start(sbrow[:], sb_d[:].unsqueeze(0))
            cwrow = cpool.tile([1, H], f32, tag="cwrow")
            nc.sync.dma_start(cwrow[:], cw_d[:].rearrange("k o -> o k"))
            tsb = cpool.tile([1, H], f32, tag="tsb")
            nc.scalar.activation(tsb[:], sbrow[:], Act.Tanh)
            nc.vector.tensor_mul(tsb[:], tsb[:], cwrow[:])
            u0p = cpool.tile([1, 1], f32, tag="u0p")
            nc.vector.reduce_sum(u0p[:], tsb[:], axis=mybir.AxisListType.X)
            u0 = cpool.tile([1, 1], f32, tag="u0")
            nc.scalar.activation(u0[:], u0p[:], Act.Tanh)
            gb_main = cpool.tile([1, 1], f32, tag="gbm")
            nc.scalar.activation(gb_main[:], u0[:], Act.Copy,
                                 bias=-math.log(A - 1.0), scale=-1.0)
            gb_root = cpool.tile([1, 1], f32, tag="gbr")
            nc.scalar.activation(gb_root[:], u0[:], Act.Copy,
                                 bias=-math.log(BS - 1.0), scale=-1.0)

            acc = cpool.tile([128, HC, TPC], f32, tag="acc")
            nc.vector.memset(acc[:], 0.0)

            h = {}
            for d in range(D + 1):
                h[d] = hpool.tile([128, HC, CNT[d]], f32, tag=f"h{d}", name=f"h{d}")

            def gather_x(d, x_tile, col0, ncols, xoff):
                """Gather 128*ncols emb rows and transpose into
                x_tile[:, :, xoff : xoff+128*ncols]  ([E-part, chunk, node])."""
                g = gpool.tile([128, 2, E], f32, tag="gath")
                # HW indirect DMA supports exactly one index per partition
                for m in range(ncols):
                    nc.gpsimd.indirect_dma_start(
                        out=g[:, m, :], out_offset=None,
                        in_=emb_d[:],
                        in_offset=bass.IndirectOffsetOnAxis(
                            ap=toks[:, col0 + m:col0 + m + 1], axis=0),
                    )
                xps = psA.tile([128, HC, TILE_N], f32, tag="psA")
                for c in range(HC):
                    for m in range(ncols):
                        nc.tensor.transpose(
                            out=xps[:, c, m * 128:(m + 1) * 128],
                            in_=g[:, m, c * 128:(c + 1) * 128],
                            identity=ident[:],
                        )
                w = 128 * ncols
                # split the PSUM->SBUF copy across ACT and DVE
                nc.scalar.copy(x_tile[:, 0, xoff:xoff + w], xps[:, 0, 0:w])
                nc.vector.tensor_copy(x_tile[:, 1, xoff:xoff + w], xps[:, 1, 0:w])

            def gru_tile(d, x_rhs, n_nodes, hslice, chs=None):
                """One GRU tile: gates = wih.T@x (+ whh.T@chs), h -> h[d][hslice].
                x_rhs: [128, HC, n] SBUF AP (emb, E-major). chs: [128, HC, n] or None.
                """
                ps_g = psG.tile([128, GC, TILE_N], f32, tag="psG")
                n_mm = 2 if chs is None else 4
                for c in range(GC):
                    rz = c < 4
                    # gi
                    for kc in range(HC):
                        nc.tensor.matmul(
                            out=ps_g[:, c, 0:n_nodes],
                            lhsT=wih[:, kc, c * 128:(c + 1) * 128],
                            rhs=x_rhs[:, kc, :],
                            start=(kc == 0),
                            stop=(kc == 1 and (chs is None or not rz)),
                        )
                    # gh accumulates for r,z chunks only
                    if chs is not None and rz:
                        for kc in range(HC):
                            nc.tensor.matmul(
                                out=ps_g[:, c, 0:n_nodes],
                                lhsT=whh[:, kc, c * 128:(c + 1) * 128],
                                rhs=chs[:, kc, 0:n_nodes],
                                start=False, stop=(kc == 1),
                            )
                ps_hn = None
                if chs is not None:
                    ps_hn = psA.tile([128, HC, TILE_N], f32, tag="psA")
                    for c in range(HC):
                        for kc in range(HC):
                            nc.tensor.matmul(
                                out=ps_hn[:, c, 0:n_nodes],
                                lhsT=whh[:, kc, (4 + c) * 128:(5 + c) * 128],
                                rhs=chs[:, kc, 0:n_nodes],
                                start=(kc == 0), stop=(kc == 1),
                            )
                rz_sb = wpool.tile([128, 4, TILE_N], f32, tag="rz")
                for c in range(4):
                    nc.scalar.activation(
                        rz_sb[:, c, 0:n_nodes], ps_g[:, c, 0:n_nodes],
                        Act.Sigmoid, bias=bsum[:, c:c + 1])
                npre = wpool.tile([128, HC, TILE_N], f32, tag="npre")
                if chs is None:
                    # n_pre = r * b_hh_n + gi_n
                    for c in range(HC):
                        nc.vector.scalar_tensor_tensor(
                            out=npre[:, c, 0:n_nodes], in0=rz_sb[:, c, 0:n_nodes],
                            scalar=bhh[:, 4 + c:5 + c], in1=ps_g[:, 4 + c, 0:n_nodes],
                            op0=Alu.mult, op1=Alu.add)
                else:
                    # n_pre = r * (gh_n + b_hh_n) + gi_n
                    rn = wpool.tile([128, HC, TILE_N], f32, tag="rn")
                    for c in range(HC):
                        nc.vector.scalar_tensor_tensor(
                            out=rn[:, c, 0:n_nodes], in0=ps_hn[:, c, 0:n_nodes],
                            scalar=bhh[:, 4 + c:5 + c], in1=rz_sb[:, c, 0:n_nodes],
                            op0=Alu.add, op1=Alu.mult)
                    nc.vector.tensor_add(
                        npre[:, :, 0:n_nodes], rn[:, :, 0:n_nodes],
                        ps_g[:, 4:6, 0:n_nodes])
                n_sb = wpool.tile([128, HC, TILE_N], f32, tag="nsb")
                for c in range(HC):
                    nc.scalar.activation(
                        n_sb[:, c, 0:n_nodes], npre[:, c, 0:n_nodes],
                        Act.Tanh, bias=bih[:, 4 + c:5 + c])
                hv = h[d][:, :, hslice]
                if chs is None:
                    # h = n - z*n
                    zn = wpool.tile([128, HC, TILE_N], f32, tag="zn")
                    nc.vector.tensor_mul(
                        zn[:, :, 0:n_nodes], rz_sb[:, 2:4, 0:n_nodes],
                        n_sb[:, :, 0:n_nodes])
                    nc.vector.tensor_tensor(
                        out=hv, in0=n_sb[:, :, 0:n_nodes],
                        in1=zn[:, :, 0:n_nodes], op=Alu.subtract)
                else:
                    # h = n + z*(chs - n)
                    dd = wpool.tile([128, HC, TILE_N], f32, tag="dd")
                    nc.vector.tensor_tensor(
                        out=dd[:, :, 0:n_nodes], in0=chs[:, :, 0:n_nodes],
                        in1=n_sb[:, :, 0:n_nodes], op=Alu.subtract)
                    zd = wpool.tile([128, HC, TILE_N], f32, tag="zd")
                    nc.vector.tensor_mul(
                        zd[:, :, 0:n_nodes], rz_sb[:, 2:4, 0:n_nodes],
                        dd[:, :, 0:n_nodes])
                    nc.vector.tensor_add(
                        hv, n_sb[:, :, 0:n_nodes], zd[:, :, 0:n_nodes])

            def attention(xc, n_nodes, gb):
                """chs = xc * sigmoid(tanh(tanh(xc@sw+sb)@cw) + gb). xc: [128,HC,n]."""
                ps_t = psA.tile([128, HC, TILE_N], f32, tag="psA")
                for mc in range(HC):
                    for kc in range(HC):
                        nc.tensor.matmul(
                            out=ps_t[:, mc, 0:n_nodes],
                            lhsT=sw[:, kc, mc * 128:(mc + 1) * 128],
                            rhs=xc[:, kc, :],
                            start=(kc == 0), stop=(kc == 1),
                        )
                t_sb = wpool.tile([128, HC, TILE_N], f32, tag="tsb_at")
                for mc in range(HC):
                    nc.scalar.activation(
                        t_sb[:, mc, 0:n_nodes], ps_t[:, mc, 0:n_nodes],
                        Act.Tanh, bias=sbc[:, mc:mc + 1])
                ps_v = psA.tile([1, TILE_N], f32, tag="psA")
                for kc in range(HC):
                    nc.tensor.matmul(
                        out=ps_v[:, 0:n_nodes], lhsT=cw[:, kc:kc + 1],
                        rhs=t_sb[:, kc, 0:n_nodes],
                        start=(kc == 0), stop=(kc == 1))
                v_sb = wpool.tile([1, TILE_N], f32, tag="vsb")
                nc.scalar.activation(v_sb[:, 0:n_nodes], ps_v[:, 0:n_nodes], Act.Tanh)
                g_sb = wpool.tile([1, TILE_N], f32, tag="gsb")
                nc.scalar.activation(g_sb[:, 0:n_nodes], v_sb[:, 0:n_nodes],
                                     Act.Sigmoid, bias=gb)
                ps_gb = psA.tile([128, TILE_N], f32, tag="psA")
                nc.tensor.matmul(out=ps_gb[:, 0:n_nodes], lhsT=ones1[:],
                                 rhs=g_sb[:, 0:n_nodes], start=True, stop=True)
                chs = wpool.tile([128, HC, TILE_N], f32, tag="chs")
                for c in range(HC):
                    nc.vector.tensor_mul(chs[:, c, 0:n_nodes], xc[:, c, :],
                                         ps_gb[:, 0:n_nodes])
                return chs

            # ---- level 6 (leaves): stream gather -> transpose -> GRU(h=0) ----
            ntiles6 = CNT[6] // TILE_N
            for i in range(ntiles6):
                x_t = xpool.tile([128, HC, TILE_N], f32, tag="x6")
                gather_x(6, x_t, COL_OFF[6] + 2 * i, 2, 0)
                gru_tile(6, x_t[:, :, :], TILE_N,
                         slice(i * TILE_N, (i + 1) * TILE_N), chs=None)

            # ---- levels 5..0 ----
            for d in range(D - 1, -1, -1):
                cnt = CNT[d]
                padded = max(128, cnt)
                x_l = xlpool.tile([128, HC, padded], f32, tag=f"x{d}", name=f"x{d}")
                k = KCOLS[d]
                for j in range(0, k, 2):
                    nj = min(2, k - j)
                    gather_x(d, x_l, COL_OFF[d] + j, nj, j * 128)
                gb = gb_root if d == 0 else gb_main
                ntiles = max(1, cnt // TILE_N)
                for i in range(ntiles):
                    n_nodes = min(TILE_N, cnt)
                    sl = slice(i * TILE_N, i * TILE_N + n_nodes)
                    if d == 0:
                        xc = h[1][:, :, :]
                    else:
                        xc = h[d + 1][:, :, 3::4][:, :, sl]
                    chs = attention(xc, n_nodes, gb[:])
                    gru_tile(d, x_l[:, :, sl], n_nodes, sl, chs=chs)

                # fold this level into the running per-tree max
                if d >= 2:
                    red = wpool.tile([128, HC, TPC], f32, tag="red")
                    nc.vector.tensor_reduce(
                        out=red[:], in_=h[d][:].rearrange(
                            "p c (t j) -> p c t j", t=TPC),
                        axis=mybir.AxisListType.X, op=Alu.max)
                    nc.vector.tensor_tensor(out=acc[:], in0=acc[:], in1=red[:],
                                            op=Alu.max)
                else:
                    nc.vector.tensor_tensor(out=acc[:], in0=acc[:], in1=h[d][:],
                                            op=Alu.max)
            # include leaves in the max
            red6 = wpool.tile([128, HC, TPC], f32, tag="red")
            nc.vector.tensor_reduce(
                out=red6[:], in_=h[6][:].rearrange("p c (t j) -> p c t j", t=TPC),
                axis=mybir.AxisListType.X, op=Alu.max)
            nc.vector.tensor_tensor(out=acc[:], in0=acc[:], in1=red6[:], op=Alu.max)

            nc.sync.dma_start(out_d[:].rearrange("c p t -> p c t"), acc[:])

    nc.compile()
    return nc


def get_program():
    global _PROG
    if _PROG is None:
        _PROG = _build_program()
    return _PROG


def shard_inputs(tokens, embedding, w_ih, w_hh, b_ih, b_hh, sent_weight,
                 sent_bias, context_weight):
    tokens = np.ascontiguousarray(np.asarray(tokens, dtype=np.int32))
    shared = {
        "emb": np.ascontiguousarray(np.asarray(embedding, dtype=np.float32)),
        "wihT": np.ascontiguousarray(np.asarray(w_ih, dtype=np.float32).T),
        "whhT": np.ascontiguousarray(np.asarray(w_hh, dtype=np.float32).T),
        "bih": np.ascontiguousarray(np.asarray(b_ih, dtype=np.float32)),
        "bhh": np.ascontiguousarray(np.asarray(b_hh, dtype=np.float32)),
        "sw": np.ascontiguousarray(np.asarray(sent_weight, dtype=np.float32)),
        "sb": np.ascontiguousarray(
            np.asarray(sent_bias, dtype=np.float32).reshape(H)),
        "cw": np.ascontiguousarray(np.asarray(context_weight, dtype=np.float32)),
    }
    return [dict(shared, toks=_token_plane(tokens, c)) for c in range(NCORES)]


LAST_RESULTS = None  # BassKernelResults of the most recent kernel() call


def kernel(**inputs):
    global LAST_RESULTS
    from concourse import bass_utils

    nc = get_program()
    in_maps = shard_inputs(**inputs)
    res = bass_utils.run_bass_kernel_spmd(nc, in_maps, core_ids=list(range(NCORES)))
    LAST_RESULTS = res
    outs = []
    for c in range(NCORES):
        o = res.results[c]["out"]  # [HC, 128, TPC]
        outs.append(np.ascontiguousarray(o.transpose(2, 0, 1).reshape(TPC, H)))
    return np.concatenate(outs, axis=0)
